# revision 1
# baseline (speedup 1.0000x reference)
"""Trainium2 Bass kernel for nn_CustomNLLLoss (binary-class NLL with per-class means).

Math: for C=2, the log_softmax picked value obeys
    -picked_i = softplus(d) if t=0 else softplus(-d) = softplus(d) - d,  d = x1 - x0
With g = softplus(d):
    sum0 = sum_{t=0} g       = S_g - S_tg
    sum1 = sum_{t=1} (g - d) = S_tg - S_td
    loss = sum0/n0 + sum1/n1
Each core produces per-partition partials of S_g, S_tg, S_td, n1; host folds.

Device mapping (per core, M = 1M samples; k in [0,128), g in [0,64), i in [0,128)):
    xI[2g+c, 128k+i] = x_c(s),  tD[i, 64k+g] = t(s),  s = 8192k + 128g + i
    PE   : d = x1 - x0 via lhsT=xI-slice [128,128] x rhs=W [128,64] (+-1 eye pair),
           64 cycles per 8192 samples; also near-free column-sum matmuls
           (lhsT=data, rhs=ones) accumulating S_g, S_tg, n1 partials in PSUM.
    ACT  : g = softplus(d) as Exp then Ln(1+e) — the real TRN2 act tables
           have no softplus; ACT is the bottleneck engine and runs gap-free.
    DVE  : q = t*d stt with accum_out => S_td partials (Pool cannot read
           PSUM on TRN2); p = t*g tensor_tensor (bf16 2x mode) summed by PE.
    POOL : only the W-matrix DMA (its 0.42-efficiency ALU is slower than
           letting DVE do everything).
    DMA  : x as fp8e4m3 (2 MiB, costs ~2e-4 rel err) + targets bf16 (2 MiB).
"""

import sys

for _p in ("/opt/trn_rl_repo", "/root/.axon_site/_ro/trn_rl_repo"):
    if _p not in sys.path:
        sys.path.append(_p)

import ml_dtypes
import numpy as np

import concourse.bass as bass
import concourse.tile as tile
from concourse import mybir
from concourse.bass_utils import run_bass_kernel_spmd

N_CORES = 8
N = 8388608
M = N // N_CORES      # samples per core (1,048,576)
P = 128               # SBUF partitions
KB = 128              # k-blocks per core; each k-block: 128 xI cols -> 64 d cols
DCOLS = KB * 64       # 8192 d-columns per core

f32 = mybir.dt.float32
bf16 = mybir.dt.bfloat16
fp8 = mybir.dt.float8e4

# Chunk sizes in k-blocks. Each chunk's d-tile is 64*nk f32 PSUM columns and
# must fit 3 banks (<=1536 cols -> nk<=24). ACT does softplus as exp+ln (no
# native softplus table on real HW), so it is the bottleneck engine: chunks
# are large to amortize its per-instruction fixed costs, with a small first
# chunk (early start) and a tiny last chunk (short tail chain).
KCH = [12, 16, 24, 24, 24, 24, 4]
# ln/p grouping: consecutive chunks share one e/g tile so the ln pass and
# the p pass run as single large instructions (ACT fixed cost amortized).
LNG = [[0], [1], [2], [3], [4], [5], [6]]
assert sum(KCH) == KB
NCH = len(KCH)
# Emit the PE column-sum matmuls for chunk i only after the sub-matmuls of
# chunk i+LAG, so the PE never stalls the next chunk on ACT/DVE outputs.
# p = t*g trails ACT by ~2 chunks, so its sums need a deeper lag.
G_LAG = 1
P_LAG = 1

# xI/t DMA slabs in k-blocks (fewer DMAs keep shared HWDGE under the
# roofline); slab boundaries must align with chunk boundaries.
XSLAB = [12, 16, 24, 24, 24, 24, 4]
TSLAB = [12, 16, 24, 24, 24, 24, 4]
assert sum(XSLAB) == KB and sum(TSLAB) == KB


def _legalize_waits(nc, max_waits=1):
    """This walrus build rejects instructions carrying more than ~1 sync
    wait ("Too many sync wait commands"), but Tile's Rust wait-assigner
    happily attaches several. Hoist excess waits onto same-engine NOPs
    inserted immediately before the instruction — sequencers execute waits
    in program order, so semantics are unchanged."""
    n = 0
    for f in nc.m.functions:
        for blk in f.blocks:
            il = blk.instructions
            i = 0
            while i < len(il):
                inst = il[i]
                si = getattr(inst, "sync_info", None)
                if si is not None and len(si.on_wait) > max_waits:
                    waits = list(si.on_wait)
                    extra, keep = waits[:-max_waits], waits[-max_waits:]
                    nops = []
                    for w in extra:
                        n += 1
                        nops.append(mybir.InstNoOp(
                            name=f"I-waitfix-{n}",
                            sync_info=mybir.SyncInfo(on_wait=[w], on_update=[]),
                            bass_nofuse=True,
                            engine=inst.engine,
                        ))
                    inst.sync_info = mybir.SyncInfo(
                        on_wait=keep, on_update=list(si.on_update)
                    )
                    il[i:i] = nops
                    i += len(nops)
                i += 1
    return nc


def build_nc():
    nc = bass.Bass("TRN2")
    xi = nc.declare_dram_parameter("xi", [P, KB * 128], fp8, isOutput=False)
    td = nc.declare_dram_parameter("td", [P, DCOLS], bf16, isOutput=False)
    w = nc.declare_dram_parameter("w", [P, 64], fp8, isOutput=False)
    # cols 0..NCH-1: DVE per-chunk sums of t*d;
    # cols NCH..NCH+2: S_g, S_tg, n1 per-partition-col accumulators
    out_all = nc.declare_dram_parameter("out_all", [P, NCH + 5], f32, isOutput=True)

    with tile.TileContext(nc) as tc:
        with (
            tc.tile_pool(name="io", bufs=4) as iop,
            tc.tile_pool(name="wk", bufs=6) as wp,
            tc.tile_pool(name="st", bufs=1) as sp,
            tc.tile_pool(name="ps", bufs=2, space="PSUM") as pp,
            tc.tile_pool(name="pa", bufs=1, space="PSUM") as pap,
        ):
            wt = sp.tile([P, 64], fp8)
            nc.gpsimd.dma_start(out=wt, in_=w[:, :])
            ones = sp.tile([P, 1], bf16)
            nc.vector.memset(ones, 1.0)
            std_a = sp.tile([P, NCH + 5], f32)
            acc = pap.tile([P, 8], f32)
            nc.vector.memset(acc, 0.0)

            # Pre-plan slab tiles: issue DMA for slab when first needed.
            xslab_tiles = [None] * len(XSLAB)
            tslab_tiles = [None] * len(TSLAB)
            xslab_off = np.cumsum([0] + XSLAB)
            tslab_off = np.cumsum([0] + TSLAB)

            def get_xslab(si):
                if xslab_tiles[si] is None:
                    k0, k1 = xslab_off[si], xslab_off[si + 1]
                    xt = iop.tile([P, (k1 - k0) * 128], fp8, tag="x")
                    nc.sync.dma_start(out=xt, in_=xi[:, k0 * 128 : k1 * 128])
                    xslab_tiles[si] = (k0, xt)
                return xslab_tiles[si]

            def get_tslab(si):
                if tslab_tiles[si] is None:
                    k0, k1 = tslab_off[si], tslab_off[si + 1]
                    tt = iop.tile([P, (k1 - k0) * 64], bf16, tag="t")
                    nc.sync.dma_start(out=tt, in_=td[:, k0 * 64 : k1 * 64])
                    tslab_tiles[si] = (k0, tt)
                return tslab_tiles[si]

            n_acc_writes = 3 * sum((nk * 64) // 128 for nk in KCH)
            acc_w = [0]
            pend_g = []  # (gt, L) awaiting PE column sums
            pend_p = []  # (pt, L)

            def emit_sums(slot, src, L):
                # near-free PE column sums: accumulate S_g, S_tg, n1 partials
                for j in range(0, L, 128):
                    acc_w[0] += 1
                    nc.tensor.matmul(
                        acc[:, slot : slot + 1],
                        lhsT=src[:, j : j + 128],
                        rhs=ones,
                        start=False,
                        stop=(acc_w[0] == n_acc_writes),
                        skip_group_check=True,
                    )

            chunk_group = {}
            for gi, members in enumerate(LNG):
                off = 0
                for m in members:
                    chunk_group[m] = (gi, off, sum(KCH[j] for j in members) * 64)
                    off += KCH[m] * 64
            group_e = [None] * len(LNG)

            koff = 0
            for ci, nk in enumerate(KCH):
                L = nk * 64
                xs_i = int(np.searchsorted(xslab_off, koff, side="right") - 1)
                ts_i = int(np.searchsorted(tslab_off, koff, side="right") - 1)
                xk0, xt = get_xslab(xs_i)
                if xs_i + 1 < len(XSLAB):
                    get_xslab(xs_i + 1)
                tk0, tt_full = get_tslab(ts_i)
                tt = tt_full[:, (koff - tk0) * 64 : (koff - tk0) * 64 + L]

                dP = pp.tile([P, L], f32, tag="d")
                for k in range(nk):
                    xo = (koff - xk0 + k) * 128
                    nc.tensor.matmul(
                        dP[:, k * 64 : (k + 1) * 64],
                        lhsT=xt[:, xo : xo + 128],
                        rhs=wt,
                        start=True,
                        stop=True,
                        skip_group_check=True,
                    )
                # deferred PE sums for older chunks: keeps the PE from
                # stalling this chunk's sub-matmuls on ACT/DVE outputs
                if len(pend_g) >= G_LAG:
                    emit_sums(0, *pend_g.pop(0))
                while len(pend_p) >= P_LAG:
                    emit_sums(1, *pend_p.pop(0))
                # n1 sums depend only on the t slab: emit as soon as the slab
                # is in flight (never stalls - DMA-dependent only)
                emit_sums(2, tt, L)

                # exp writes into the group's shared e-tile; the ln (and p)
                # for the whole group run as one instruction each once the
                # last member chunk's exp is emitted.
                gi, goff, glen = chunk_group[ci]
                if group_e[gi] is None:
                    group_e[gi] = wp.tile([P, glen], bf16, tag="e", name=f"e{gi}")
                et = group_e[gi]
                nc.scalar.activation(
                    out=et[:, goff : goff + L], in_=dP,
                    func=mybir.ActivationFunctionType.Exp,
                )
                # q = t*d: DVE only (Pool cannot read PSUM on TRN2)
                qd = wp.tile([P, L], bf16, tag="qd")
                nc.vector.scalar_tensor_tensor(
                    out=qd, in0=tt, scalar=1.0, in1=dP,
                    op0=mybir.AluOpType.mult, op1=mybir.AluOpType.mult,
                    accum_out=std_a[:, ci : ci + 1],
                )
                if ci == LNG[gi][-1]:
                    tg = tt_full[:, (koff + nk - tk0) * 64 - glen :
                                 (koff + nk - tk0) * 64]
                    gt = wp.tile([P, glen], bf16, tag="g")
                    last_g = gi == len(LNG) - 1
                    nc.scalar.activation(
                        out=gt, in_=et, func=mybir.ActivationFunctionType.Ln,
                        bias=1.0, scale=1.0,
                    )
                    # p = t*g on DVE (tensor_tensor runs at 2x for packed
                    # bf16). The last group uses stt with accum_out instead:
                    # one hop shorter at the tail (no PE sum -> copy chain).
                    pt = wp.tile([P, glen], bf16, tag="p")
                    if gi == len(LNG) - 1:
                        nc.vector.scalar_tensor_tensor(
                            out=pt, in0=tg, scalar=1.0, in1=gt,
                            op0=mybir.AluOpType.mult, op1=mybir.AluOpType.mult,
                            accum_out=std_a[:, NCH + 3 : NCH + 4],
                        )
                    else:
                        nc.vector.tensor_tensor(
                            out=pt, in0=tg, in1=gt, op=mybir.AluOpType.mult
                        )
                        pend_p.append((pt, glen))
                    pend_g.append((gt, glen))
                koff += nk

            while pend_g:
                emit_sums(0, *pend_g.pop(0))
            while pend_p:
                emit_sums(1, *pend_p.pop(0))

            nc.scalar.copy(out=std_a[:, NCH : NCH + 3], in_=acc[:, 0:3])
            nc.sync.dma_start(out=out_all[:, :], in_=std_a[:, :])
    return _legalize_waits(nc)


def _strip_second_barrier(nc):
    """TileContext exit emits two all-engine barrier rounds back to back; the
    second is redundant for this kernel (no engine issues work after the
    first) and costs ~0.4us of pure epilogue. Drop everything after the
    first complete barrier round."""
    for f in nc.m.functions:
        for blk in f.blocks:
            il = blk.instructions
            # find the LAST Drain on SP that waits on a DMA queue (end of the
            # real program), then the first full barrier round after it;
            # truncate after that round's final instruction.
            idx = None
            for i, inst in enumerate(il):
                if (inst.opcode == 'Drain'
                        and str(inst.engine) == 'EngineType.SP'
                        and inst.sync_info is not None
                        and any('DMAHW' in w.ant_name or 'DMASW' in w.ant_name
                                for w in inst.sync_info.on_wait)):
                    idx = i
            if idx is not None:
                del il[idx + 1 :]
    return nc


_NC = None


def get_nc():
    global _NC
    if _NC is None:
        _NC = build_nc()
    return _NC


def _make_w():
    w = np.zeros((P, 64), dtype=np.float32)
    for g in range(64):
        w[2 * g, g] = -1.0
        w[2 * g + 1, g] = 1.0
    return w.astype(ml_dtypes.float8_e4m3)


def run_device(x, t, **spmd_kwargs):
    """x: [N,2] f32, t: [N] int. Returns (S_g, S_tg, S_td, n1) float64 + results."""
    wm = _make_w()
    in_maps = []
    for c in range(N_CORES):
        xs = x[c * M : (c + 1) * M]
        ts = t[c * M : (c + 1) * M]
        xv = xs.reshape(KB, 64, 128, 2)                       # [k, g, i, c]
        xI = np.ascontiguousarray(xv.transpose(1, 3, 0, 2)).reshape(P, KB * 128)
        tv = ts.reshape(KB, 64, 128)                          # [k, g, i]
        tD = np.ascontiguousarray(tv.transpose(2, 0, 1)).reshape(P, DCOLS)
        in_maps.append({
            "xi": xI.astype(ml_dtypes.float8_e4m3),
            "td": tD.astype(ml_dtypes.bfloat16),
            "w": wm,
        })
    res = run_bass_kernel_spmd(get_nc(), in_maps, list(range(N_CORES)), **spmd_kwargs)
    s_td = np.float64(0.0)
    s_g = s_tg = n1 = np.float64(0.0)
    for r in res.results:
        a = r["out_all"].astype(np.float64)
        s_td += a[:, 0:NCH].sum()
        s_g += a[:, NCH].sum() + a[:, NCH + 4].sum()
        s_tg += a[:, NCH + 1].sum() + a[:, NCH + 3].sum()
        n1 += a[:, NCH + 2].sum()
    return (s_g, s_tg, s_td, n1), res


def kernel(x, targets):
    x = np.ascontiguousarray(np.asarray(x), dtype=np.float32)
    t = np.ascontiguousarray(np.asarray(targets))
    (s_g, s_tg, s_td, n1), _ = run_device(x, t)
    sum0 = s_g - s_tg
    sum1 = s_tg - s_td
    n0 = float(N) - n1
    p = sum0 / n0 if n0 > 0 else 0.0
    r = sum1 / n1 if n1 > 0 else 0.0
    return np.array(p + r, dtype=np.float32)

